# revision 4
# baseline (speedup 1.0000x reference)
"""Trainium2 Bass kernel for GNN message passing — v6 (shared-target aligned
ragged tiling).

    out = segment_sum(x[src] @ W, tgt, N) + x @ W_self
       = segment_sum(x[src], tgt) @ W + x @ W_self

v3 padded every target window to a uniform t_win=8 tiles (1024 edge slots vs
816 avg) because the gather/S/matmul structure was window-uniform. The kernel
is entirely Q7 descriptor-generation bound (~2.12 ns/idx aggregate over the 4
SWDGE queues, measured), so slot count is the cost. v4 packs each gather call
(group of G_WIN windows x chunk) with the windows' runs CONCATENATED, padding
only to the call's 128-slot tile boundary, with a cross-core max profile so
all 8 cores share one SPMD program:

  - call (g,c) has m_gc = max_core ceil(L_gc/128) tiles; per-core idx arrays
    are 0-padded (row-0 gathers, tl=-1) to the shared profile;
  - window boundaries fall mid-tile and differ per core; each window's matmul
    covers the tile span [a,b] = min/max over cores of its run's tiles, with
    per-core tl = -1 masking foreign slots (S column = 0);
  - a boundary tile is consumed by both adjacent windows with complementary
    masks (one extra 128^3 matmul per boundary; PE has headroom).

v6: chunks got SHARED (cross-core) 128-multiple run targets with chunk 3
absorbing remainders ragged (96k slots, 859 uses, 219us).

v7 (current): the ALIGNED chunks are the two END chunks (0 and 3), whose
natural per-window coverage (~267 edges over their 32768-row spans) sits just
above the 256 target, so alignment is nearly pad-free; the LIGHT middle
chunks (1, 2; must-load ~98/window) absorb overflow ragged and pool across
the group into few tiles. Every window is aligned in c0/c3 (boundaries
identical on all cores -> single-window tiles); middle-chunk boundary tiles
are double-consumed with complementary tl masks. 85,248 slots/core, 802
tile-use matmuls. The per-window throwaway sync matmul is dropped (Bacc
legalizes the multi-wait matmuls). Measured ~6us faster than v6 under
identical conditions (226.6 vs 232.4 on a degraded device; v6's clean-device
figure was 219.3).
"""

import numpy as np

P = 128
D = 128
N_NODES = 100000
N_CORES = 8
N_LOC = N_NODES // N_CORES          # 12500
N_WIN = (N_LOC + P - 1) // P        # 98
N_PAD = N_WIN * P                   # 12544

N_CHUNK = 4
CHUNK_SPAN = 32768
CHUNK_BASE = [0, 22411, 44822, N_NODES - CHUNK_SPAN]

G_WIN = 3
MAX_CALL_TILES = 7                  # 896 idx < 1008 SWDGE ring cap

_program_cache: dict = {}


def _group_sizes(g_win=G_WIN):
    sizes = [g_win] * (N_WIN // g_win)
    if N_WIN % g_win:
        sizes.append(N_WIN % g_win)
    return sizes


def _build_program(profile, reps: int = 1, w_group: int = 4, g_bufs: int = 8,
                   psum_bufs: int = 4, spool_bufs: int = 6, use_scratch: bool = False):
    """profile: dict with
    m_gc[g][c]: tiles per (group, chunk) call;
    uses[w]: list of (c, j) tile coords for window w's matmuls (j local to
             the (g,c) call region);
    u_max: max len(uses[w]);
    tl_off[w]: column offset of window w's tl block (each use = 128 slots,
             but tl is stored one bf16 column of 128 partitions per use);
    """
    import concourse.mybir as mybir
    import concourse.tile as tile
    from concourse.bacc import Bacc

    f32 = mybir.dt.float32
    bf16 = mybir.dt.bfloat16

    m_gc = profile["m_gc"]
    uses = profile["uses"]
    u_max = profile["u_max"]
    sizes = _group_sizes()
    n_groups = len(sizes)

    # per-(g,c) idx16 column offsets (128 idx = 8 int16 cols per tile)
    idx_off = {}
    off = 0
    for g in range(n_groups):
        for c in range(N_CHUNK):
            idx_off[(g, c)] = off
            off += m_gc[g][c] * 8
    idx_cols16 = off
    idx_cols = idx_cols16 // 2

    n_uses_tot = sum(len(u) for u in uses)
    tl_cols = (n_uses_tot + 1) // 2          # bf16 cols -> int32 cols
    iota_cols = u_max * P // 2
    w_cols = D // 2
    k_const = idx_cols + tl_cols + iota_cols + 2 * w_cols

    nc = Bacc(num_swdge_queues=N_CHUNK)
    xb_d = nc.declare_dram_parameter("xb", [N_NODES, D], bf16, isOutput=False)
    xT_d = nc.declare_dram_parameter("xT", [D, N_PAD], bf16, isOutput=False)
    consts_d = nc.declare_dram_parameter(
        "consts", [P, k_const], mybir.dt.int32, isOutput=False
    )
    outT_d = nc.declare_dram_parameter("outT", [D, N_PAD], bf16, isOutput=True)

    with tile.TileContext(nc) as tc:
        with (
            tc.tile_pool(name="const", bufs=1) as cpool,
            tc.tile_pool(name="gath", bufs=g_bufs) as gpool,
            tc.tile_pool(name="spool", bufs=spool_bufs) as spool,
            tc.tile_pool(name="wtile", bufs=3) as wpool,
            tc.tile_pool(name="psum", bufs=psum_bufs, space="PSUM") as psum,
            tc.tile_pool(name="opsum", bufs=2, space="PSUM") as opsum,
            tc.tile_pool(name="scratch", bufs=1, space="PSUM") as scratch_pool,
        ):
            scratch_ps = scratch_pool.tile([1, 1], f32)
            idx_t = cpool.tile([P, idx_cols], mybir.dt.int32)
            head16 = idx_off[(min(8, n_groups - 1), 0)]
            head = head16 // 2
            if head > 0:
                nc.sync.dma_start(idx_t[:, :head], consts_d[:, :head])
                nc.sync.dma_start(idx_t[:, head:], consts_d[:, head:idx_cols])
            else:
                nc.sync.dma_start(idx_t[:], consts_d[:, :idx_cols])
            rest_t = cpool.tile([P, k_const - idx_cols], mybir.dt.int32)
            nc.sync.dma_start(rest_t[:], consts_d[:, idx_cols:])
            idx16_sb = idx_t[:].bitcast(mybir.dt.int16)
            xT_sb = cpool.tile([D, N_PAD], bf16)
            nc.sync.dma_start(xT_sb[:], xT_d[:])

            o = 0
            tl_sb = rest_t[:, o : o + tl_cols].bitcast(bf16)
            o += tl_cols
            iota_sb = rest_t[:, o : o + iota_cols].bitcast(bf16)
            o += iota_cols
            w_sb = rest_t[:, o : o + w_cols].bitcast(bf16)
            o += w_cols
            ws_sb = rest_t[:, o : o + w_cols].bitcast(bf16)
            # iota_nt[p, n, u] = n (constant along u)
            iota_nt = iota_sb.rearrange("p (n u) -> p n u", u=u_max)

            tl_off = profile["tl_off"]

            # queue = chunk: per-group distinct queues in a fixed emission
            # order. Rebalancing queue assignment (greedy or per-group
            # bijective) measured neutral-to-unstable — the fixed pattern is
            # the reliable one even though chunk loads are uneven.
            qmap = {(g, c): c for g in range(len(sizes)) for c in range(N_CHUNK)}

            for rep in range(reps):
                w0 = 0
                for g, g_sz in enumerate(sizes):
                    m_g = sum(m_gc[g])
                    # tile j of chunk c lives at G_big[:, goff[c] + j, :]
                    goff = np.cumsum([0] + list(m_gc[g]))[:-1]
                    G_big = gpool.tile([P, m_g, D], bf16, tag="G")
                    for c in range(N_CHUNK):
                        mt = m_gc[g][c]
                        for t0 in range(0, mt, MAX_CALL_TILES):
                            t1 = min(t0 + MAX_CALL_TILES, mt)
                            nidx = (t1 - t0) * P
                            c0 = idx_off[(g, c)] + t0 * 8
                            nc.gpsimd.dma_gather(
                                G_big[:, goff[c] + t0 : goff[c] + t1, :],
                                xb_d[CHUNK_BASE[c] : CHUNK_BASE[c] + CHUNK_SPAN, :],
                                idx16_sb[:, c0 : c0 + (t1 - t0) * 8],
                                nidx,
                                nidx,
                                D,
                                queue_num=qmap[(g, c)],
                            )
                    for wl in range(g_sz):
                        w = w0 + wl
                        w_uses = uses[w]
                        nu = len(w_uses)
                        hT_ps = psum.tile([D, P], f32)
                        S_big = spool.tile([P, P, u_max], bf16, tag="S")
                        nc.vector.tensor_tensor(
                            out=S_big[:, :, 0:nu],
                            in0=iota_nt[:, :, 0:nu],
                            in1=tl_sb[
                                :, None, tl_off[w] : tl_off[w] + nu
                            ].to_broadcast([P, P, nu]),
                            op=mybir.AluOpType.is_equal,
                        )
                        # 1x1 throwaway matmul: makes the PE queue observe the
                        # DVE tick first so each real matmul carries one wait
                        if use_scratch:
                            nc.tensor.matmul(
                                scratch_ps[:],
                                lhsT=S_big[:, 0, 0:1],
                                rhs=S_big[:, 0, 0:1],
                                start=True,
                                stop=True,
                            )
                        for u, (c, j) in enumerate(w_uses):
                            nc.tensor.matmul(
                                hT_ps[:],
                                lhsT=G_big[:, goff[c] + j, :],
                                rhs=S_big[:, :, u],
                                start=(u == 0),
                                stop=(u == nu - 1),
                            )
                        gi = w % w_group
                        if gi == 0:
                            n_in_grp = min(w_group, N_WIN - w)
                            hT_sb = wpool.tile([D, w_group * P], bf16, tag="hT")
                        nc.scalar.copy(hT_sb[:, gi * P : (gi + 1) * P], hT_ps[:])
                        if gi == n_in_grp - 1:
                            wg0 = w - gi
                            span = n_in_grp * P
                            outT_ps = opsum.tile([D, w_group * P], f32)
                            nc.tensor.matmul(
                                outT_ps[:, :span],
                                lhsT=w_sb,
                                rhs=hT_sb[:, :span],
                                start=True,
                                stop=False,
                            )
                            nc.tensor.matmul(
                                outT_ps[:, :span],
                                lhsT=ws_sb,
                                rhs=xT_sb[:, wg0 * P : wg0 * P + span],
                                start=False,
                                stop=True,
                            )
                            o_sb = wpool.tile([D, w_group * P], bf16, tag="o")
                            nc.scalar.copy(o_sb[:, :span], outT_ps[:, :span])
                            nc.sync.dma_start(
                                outT_d[:, wg0 * P : wg0 * P + span],
                                o_sb[:, :span],
                            )
                    w0 += g_sz

    nc.finalize()
    return nc


def _prep_inputs(x, edge_index, W, W_self):
    """Host-side: sort edges by target window, balance chunks per group,
    build the shared cross-core profile + per-core const tensors."""
    import ml_dtypes

    x = np.ascontiguousarray(np.asarray(x, dtype=np.float32))
    W = np.ascontiguousarray(np.asarray(W, dtype=np.float32))
    W_self = np.ascontiguousarray(np.asarray(W_self, dtype=np.float32))
    ei = np.asarray(edge_index)
    src = ei[0].astype(np.int64)
    tgt = ei[1].astype(np.int64)

    order = np.argsort(tgt, kind="stable")
    src_s = src[order]
    tgt_s = tgt[order]
    core = tgt_s // N_LOC
    wloc = (tgt_s - core * N_LOC) // P
    gw = (core * N_WIN + wloc).astype(np.int64)
    counts = np.bincount(gw, minlength=N_CORES * N_WIN)
    starts = np.concatenate([[0], np.cumsum(counts)])
    tl_val = (tgt_s - (core * N_LOC + wloc * P)).astype(np.float32)

    bases = np.asarray(CHUNK_BASE, np.int64)
    # feasible chunk range [hi, lo] (consecutive), as in v3
    lo = np.searchsorted(bases, src_s, side="right") - 1
    hi = np.searchsorted(bases + CHUNK_SPAN, src_s, side="right")

    sizes = _group_sizes()
    n_groups = len(sizes)

    # ---- per (core, group): assign edges to chunks, balanced ----
    # run_idx[core][g][c] = int16 idx array (concatenated windows, in window
    # order); run_tl[core][g][c] = matching tl floats;
    # run_bounds[core][g][c] = cumulative slot starts per window (len g_sz+1)
    run_idx = [[[None] * N_CHUNK for _ in range(n_groups)] for _ in range(N_CORES)]
    run_tl = [[[None] * N_CHUNK for _ in range(n_groups)] for _ in range(N_CORES)]
    run_bounds = np.zeros((N_CORES, n_groups, N_CHUNK, G_WIN + 1), np.int64)

    gstarts = np.concatenate([[0], np.cumsum(sizes)])
    # Shared-target alignment: for every window except the last of its group,
    # chunks 0..2 get a SHARED (cross-core) run length that is a multiple of
    # 128. Each core fills must-edges + carried flex + pulled flex, then pads
    # with 0-idx/-1-tl. Aligned boundaries are identical on all 8 cores, so
    # boundary tiles are consumed by exactly one window and use-spans don't
    # widen. Chunk 3 takes each window's remainder (ragged).
    for g, g_sz in enumerate(sizes):
        w_lo = gstarts[g]
        # pass 1: per (core, window): must edge lists per chunk, flex per pair
        musts = [[None] * g_sz for _ in range(N_CORES)]
        flexs = [[None] * g_sz for _ in range(N_CORES)]
        for cc in range(N_CORES):
            for wl in range(g_sz):
                gidx = cc * N_WIN + w_lo + wl
                a, b = starts[gidx], starts[gidx + 1]
                s_g = src_s[a:b]
                hi_g, lo_g = hi[a:b], lo[a:b]
                t_g = tl_val[a:b]
                is_flex = hi_g < lo_g
                musts[cc][wl] = [
                    (s_g[m], t_g[m])
                    for m in [np.where(~is_flex & (np.minimum(hi_g, lo_g) == c))[0]
                              for c in range(N_CHUNK)]
                ]
                flexs[cc][wl] = [
                    (s_g[m], t_g[m])
                    for m in [np.where(is_flex & (hi_g == q))[0] for q in range(3)]
                ]
                # each entry is an (s_array, tl_array) pair; nlen() below
        def nlen(pair):
            return int(pair[0].size)

        # pass 2: shared 128-multiple targets for the END chunks (0 and 3),
        # whose natural per-window coverage (~267 edges) sits just above 256
        # so alignment is nearly pad-free. The LIGHT middle chunks (1, 2)
        # absorb overflow ragged: their per-window loads (~150) pool across
        # the group into few tiles. Every window is aligned in c0/c3.
        T = np.zeros((g_sz, 2), np.int64)  # [:, 0] -> chunk0, [:, 1] -> chunk3
        for wl in range(g_sz):
            m0 = np.array([nlen(musts[cc][wl][0]) for cc in range(N_CORES)])
            m3 = np.array([nlen(musts[cc][wl][3]) for cc in range(N_CORES)])
            T[wl, 0] = 128 * max(1, -(-int(m0.max()) // 128))
            T[wl, 1] = 128 * max(1, -(-int(m3.max()) // 128))
        # pass 3: materialize per-core runs
        for cc in range(N_CORES):
            per_chunk_idx = [[] for _ in range(N_CHUNK)]
            per_chunk_tl = [[] for _ in range(N_CHUNK)]

            def put(c, pair):
                s, t = pair
                per_chunk_idx[c].extend((np.asarray(s) - bases[c]).tolist())
                per_chunk_tl[c].extend(np.asarray(t).tolist())

            def pad(c, n):
                per_chunk_idx[c].extend([0] * n)
                per_chunk_tl[c].extend([-1.0] * n)

            for wl in range(g_sz):
                mu, fl = musts[cc][wl], flexs[cc][wl]
                # chunk 0: fill to T0 with must0 + flex01, pad remainder
                take0 = min(nlen(fl[0]), max(0, int(T[wl, 0]) - nlen(mu[0])))
                put(0, mu[0])
                put(0, (fl[0][0][:take0], fl[0][1][:take0]))
                pad(0, int(T[wl, 0]) - nlen(mu[0]) - take0)
                left01 = (fl[0][0][take0:], fl[0][1][take0:])
                # chunk 3: fill to T3 with must3 + flex23, pad remainder
                take3 = min(nlen(fl[2]), max(0, int(T[wl, 1]) - nlen(mu[3])))
                put(3, mu[3])
                put(3, (fl[2][0][:take3], fl[2][1][:take3]))
                pad(3, int(T[wl, 1]) - nlen(mu[3]) - take3)
                left23 = (fl[2][0][take3:], fl[2][1][take3:])
                # middles ragged: c1 = must1+left01, c2 = must2+left23,
                # flex12 split to balance the two calls' cumulative loads
                put(1, mu[1])
                put(1, left01)
                put(2, mu[2])
                put(2, left23)
                l1 = len(per_chunk_idx[1])
                l2 = len(per_chunk_idx[2])
                xs = max(0, min(nlen(fl[1]), (nlen(fl[1]) + l2 - l1) // 2))
                put(1, (fl[1][0][:xs], fl[1][1][:xs]))
                put(2, (fl[1][0][xs:], fl[1][1][xs:]))
                for c in range(N_CHUNK):
                    run_bounds[cc, g, c, wl + 1] = len(per_chunk_idx[c])
            for c in range(N_CHUNK):
                run_idx[cc][g][c] = np.asarray(per_chunk_idx[c], np.int16)
                run_tl[cc][g][c] = np.asarray(per_chunk_tl[c], np.float32)

    # ---- shared profile ----
    m_gc = [[0] * N_CHUNK for _ in range(n_groups)]
    for g in range(n_groups):
        for c in range(N_CHUNK):
            mx = max(len(run_idx[cc][g][c]) for cc in range(N_CORES))
            m_gc[g][c] = max(1, (int(mx) + P - 1) // P)

    # window uses: span of tiles [a,b] over cores for each (w, c)
    uses = []
    for g, g_sz in enumerate(sizes):
        for wl in range(g_sz):
            w_uses = []
            for c in range(N_CHUNK):
                a_t, b_t = None, None
                for cc in range(N_CORES):
                    s0 = run_bounds[cc, g, c, wl]
                    s1 = run_bounds[cc, g, c, wl + 1]
                    if s1 > s0:
                        ta = int(s0 // P)
                        tb = int((s1 - 1) // P)
                        a_t = ta if a_t is None else min(a_t, ta)
                        b_t = tb if b_t is None else max(b_t, tb)
                if a_t is not None:
                    for j in range(a_t, b_t + 1):
                        w_uses.append((c, j))
            uses.append(w_uses)
    u_max = max(len(u) for u in uses)
    tl_off = np.concatenate([[0], np.cumsum([len(u) for u in uses])])[:-1]
    n_uses_tot = int(sum(len(u) for u in uses))

    profile = {
        "m_gc": tuple(tuple(r) for r in m_gc),
        "uses": tuple(tuple(u) for u in uses),
        "u_max": u_max,
        "tl_off": tuple(int(t) for t in tl_off),
    }

    # ---- per-core const tensors ----
    idx_cols16 = sum(m_gc[g][c] * 8 for g in range(n_groups) for c in range(N_CHUNK))
    tl_cols16 = 2 * ((n_uses_tot + 1) // 2)
    iota_nu = np.tile(
        np.repeat(np.arange(P, dtype=np.float32), u_max).astype(ml_dtypes.bfloat16),
        (P, 1),
    )
    x_b = x.astype(ml_dtypes.bfloat16)
    W_b = W.astype(ml_dtypes.bfloat16)
    Ws_b = W_self.astype(ml_dtypes.bfloat16)

    in_maps = []
    for cc in range(N_CORES):
        idx_parts = []
        for g in range(n_groups):
            for c in range(N_CHUNK):
                cap = m_gc[g][c] * P
                arr = np.zeros(cap, np.int16)
                r = run_idx[cc][g][c]
                arr[: len(r)] = r
                idx_parts.append(arr.reshape(-1, 16))
        wrapped = np.concatenate(idx_parts, axis=0)
        idx_sb = np.tile(wrapped.T.reshape(16, -1), (8, 1))  # [128, idx_cols16]

        tl_flat = np.full((n_uses_tot, P), -1.0, np.float32)
        for g, g_sz in enumerate(sizes):
            w_lo = gstarts[g]
            for wl in range(g_sz):
                w = w_lo + wl
                for u, (c, j) in enumerate(profile["uses"][w]):
                    s0 = run_bounds[cc, g, c, wl]
                    s1 = run_bounds[cc, g, c, wl + 1]
                    t_lo = j * P
                    # slots of tile j that belong to this window
                    lo_s = max(s0, t_lo)
                    hi_s = min(s1, t_lo + P)
                    if hi_s > lo_s:
                        vals = run_tl[cc][g][c][lo_s:hi_s]
                        tl_flat[tl_off[w] + u, lo_s - t_lo : hi_s - t_lo] = vals
        # [P, n_uses] with pad column to even count
        tl_sb = np.full((P, tl_cols16), -1.0, np.float32)
        tl_sb[:, :n_uses_tot] = tl_flat.T
        tl_sb = tl_sb.astype(ml_dtypes.bfloat16)

        xT_c = np.zeros((D, N_PAD), np.float32)
        xT_c[:, :N_LOC] = x[cc * N_LOC : (cc + 1) * N_LOC].T
        consts = np.concatenate(
            [
                idx_sb.view(np.int32),
                tl_sb.view(np.int32),
                iota_nu.view(np.int32),
                W_b.view(np.int32),
                Ws_b.view(np.int32),
            ],
            axis=1,
        )
        in_maps.append(
            {
                "xb": x_b,
                "xT": xT_c.astype(ml_dtypes.bfloat16),
                "consts": consts,
            }
        )
    return in_maps, profile


def run(x, edge_index, W, W_self, trace=False, **trace_kwargs):
    from concourse import bass_utils

    in_maps, profile = _prep_inputs(x, edge_index, W, W_self)
    key = (profile["m_gc"], profile["uses"])
    nc = _program_cache.get(key)
    if nc is None:
        nc = _build_program(profile)
        _program_cache[key] = nc
    try:
        res = bass_utils.run_bass_kernel_spmd(
            nc, in_maps, core_ids=list(range(N_CORES)), trace=trace, **trace_kwargs
        )
    except Exception:
        res = bass_utils.run_bass_kernel_spmd(
            nc, in_maps, core_ids=list(range(N_CORES)), trace=trace, **trace_kwargs
        )
    out = np.empty((N_NODES, D), np.float32)
    for c in range(N_CORES):
        out[c * N_LOC : (c + 1) * N_LOC] = (
            res.results[c]["outT"].astype(np.float32).T[:N_LOC]
        )
    return out, res


def kernel(x, edge_index, W, W_self):
    out, _ = run(x, edge_index, W, W_self, trace=False)
    return out
